# revision 3
# baseline (speedup 1.0000x reference)
"""Block-circulant process, frequency-domain factorization, 8 cores, bf16.

v4: single batch chunk; DRAM bounce with scatter-on-WRITE (posted writes
absorb the strided side) and fully-contiguous reads split in e/i-quarters
so stage M/C matmuls start as soon as the first quarter lands. F+G load
first (tiny) so stage A starts immediately; Wmid loads later, hidden
under stage A. PSUM->SBUF copies rotate over vector/scalar/gpsimd.
"""

import numpy as np
import ml_dtypes

B = 128
K_HALF = B // 2 + 1
KT = 48
KI = 32
KO = 32
BATCH = 4096
IN_F = 4096
OUT_F = 4096

N_CORES = 8
BQ = BATCH // N_CORES  # 512
NP = KT // 2  # 24

WCOL_W = NP * 128
WCOL_FG = 96 + 128

JG = 8

_CACHE = {}
LAST_RESULTS = None
TRACE = False


def _build_nc():
    import concourse.bacc as bacc
    import concourse.mybir as mybir
    import concourse.tile as tile

    BF16 = mybir.dt.bfloat16
    F32 = mybir.dt.float32

    nc = bacc.Bacc(None, target_bir_lowering=False)
    xT = nc.declare_dram_parameter("xT", [IN_F, BQ], BF16, isOutput=False)
    wfg = nc.declare_dram_parameter("wfg", [128, WCOL_FG], BF16,
                                    isOutput=False)
    wmid = nc.declare_dram_parameter("wmid", [128, WCOL_W], BF16,
                                     isOutput=False)
    oT = nc.declare_dram_parameter("oT", [OUT_F, BQ], BF16, isOutput=True)

    # bounce buffers laid out READ-optimally (reads fully contiguous);
    # writes scatter into them
    sD = nc.dram_tensor("sD", [128, NP * BQ], BF16)   # rows (fp j), cols (e b)
    cD = nc.dram_tensor("cD", [96, KO * BQ], BF16)    # rows (fq e), cols (i b)
    sD_v = sD.rearrange("(fp j) (e b) -> fp j e b", fp=4, e=NP)
    cD_v = cD.rearrange("(fq e) (i b) -> fq e i b", fq=4, i=KO)

    xT_v = xT.rearrange("(j t) b -> t j b", t=128)
    oT_v = oT.rearrange("(i t) b -> t i b", t=128)

    JH = KI // 2   # j-half for r1 writes
    EH = NP // 2   # e-half for r2 writes
    EQ = NP // 4   # e-quarter for r1 reads
    IQ = KO // 4   # i-quarter for r2 reads

    with tile.TileContext(nc) as tc:
        with (
            tc.tile_pool(name="cpool", bufs=1) as cpool,
            tc.tile_pool(name="xpool", bufs=1) as xpool,
            tc.tile_pool(name="spool", bufs=1) as spool,
            tc.tile_pool(name="mpool", bufs=1) as mpool,
            tc.tile_pool(name="midpool", bufs=1) as midpool,
            tc.tile_pool(name="crpool", bufs=1) as crpool,
            tc.tile_pool(name="opool", bufs=1) as opool,
            tc.tile_pool(name="psum", bufs=8, space="PSUM") as psum,
        ):
            x_t = xpool.tile([128, KI * BQ], BF16, name="x")
            S_t = spool.tile([96, KI * BQ], BF16, name="S")
            m_t = mpool.tile([128, NP * BQ], BF16, name="m")
            mid_t = midpool.tile([128, NP * BQ], BF16, name="mid")
            c_t = crpool.tile([96, KO * BQ], BF16, name="c")
            o_t = opool.tile([128, KO * BQ], BF16, name="o")

            # tiny F+G first so stage A can start right away
            fg = cpool.tile([128, WCOL_FG], BF16, name="fg")
            nc.sync.dma_start(fg[:], wfg[:])
            f_t = fg[:, 0:96]
            g_t = fg[0:96, 96:WCOL_FG]

            xv = x_t.rearrange("t (j b) -> t j b", j=KI)
            for g in range(KI // JG):
                nc.sync.dma_start(xv[:, g * JG:(g + 1) * JG, :],
                                  xT_v[:, g * JG:(g + 1) * JG, :])

            def copy(eng, dst, src):
                if eng is nc.scalar:
                    eng.copy(dst, src)
                else:
                    eng.tensor_copy(dst, src)

            engs = [nc.vector, nc.scalar]

            JQ = KI // 4

            def r1_write(jq, last):
                js = slice(jq * JQ, (jq + 1) * JQ)
                cols = slice(jq * JQ * BQ, (jq + 1) * JQ * BQ)
                for fp in range(4):
                    eng = nc.scalar if (last and fp % 2 == 0) else nc.gpsimd
                    eng.dma_start(
                        sD_v[fp, js].rearrange("j e b -> e j b"),
                        S_t[fp * 24:(fp + 1) * 24, cols])

            # ---- stage A + r1 scattered writes (quarter flushes) ----
            for j in range(KI):
                ps = psum.tile([128, BQ], F32, name="ps", tag="ps")
                nc.tensor.matmul(ps[0:96, :], f_t,
                                 x_t[:, j * BQ:(j + 1) * BQ],
                                 start=True, stop=True)
                copy(engs[j % 2], S_t[:, j * BQ:(j + 1) * BQ], ps[0:96, :])
                if j == JH - 1:
                    # Wmid load rides sync after x, before M needs it
                    w_t = cpool.tile([128, WCOL_W], BF16, name="w")
                    nc.sync.dma_start(w_t[:], wmid[:])
                if j % JQ == JQ - 1:
                    r1_write(j // JQ, j == KI - 1)

            # r1 reads: contiguous e-quarters, issue spread sync/scalar
            for q in range(4):
                cols = slice(q * EQ * BQ, (q + 1) * EQ * BQ)
                (nc.sync if q % 2 == 0 else nc.scalar).dma_start(
                    m_t[:, cols], sD[:, cols])

            # ---- stage M + r2 scattered writes (quarter flushes) ----
            EQW = NP // 4

            def r2_write(eq, last):
                es = slice(eq * EQW, (eq + 1) * EQW)
                cols = slice(eq * EQW * BQ, (eq + 1) * EQW * BQ)
                for fq in range(4):
                    eng = nc.scalar if (last and fq % 2 == 0) else nc.gpsimd
                    eng.dma_start(
                        cD_v[fq, es].rearrange("e i b -> i e b"),
                        mid_t[fq * 32:(fq + 1) * 32, cols])

            for e in range(NP):
                ps = psum.tile([128, BQ], F32, name="ps", tag="ps")
                nc.tensor.matmul(ps[:], w_t[:, e * 128:(e + 1) * 128],
                                 m_t[:, e * BQ:(e + 1) * BQ],
                                 start=True, stop=True)
                copy(engs[e % 2], mid_t[:, e * BQ:(e + 1) * BQ], ps[:])
                if e % EQW == EQW - 1:
                    r2_write(e // EQW, e == NP - 1)

            # r2 reads: contiguous i-quarters, issue spread sync/scalar
            for q in range(4):
                cols = slice(q * IQ * BQ, (q + 1) * IQ * BQ)
                (nc.sync if q % 2 == 0 else nc.scalar).dma_start(
                    c_t[:, cols], cD[:, cols])

            # ---- stage C + out stores ----
            ov = o_t.rearrange("t (i b) -> t i b", i=KO)
            for i in range(KO):
                ps = psum.tile([128, BQ], F32, name="ps", tag="ps")
                nc.tensor.matmul(ps[:], g_t, c_t[:, i * BQ:(i + 1) * BQ],
                                 start=True, stop=True)
                copy(engs[i % 2], o_t[:, i * BQ:(i + 1) * BQ], ps[:])
                if i % JG == JG - 1:
                    g = i // JG
                    nc.sync.dma_start(
                        oT_v[:, g * JG:(g + 1) * JG, :],
                        ov[:, g * JG:(g + 1) * JG, :])
    nc.finalize()
    return nc


def _get_nc():
    if "nc" not in _CACHE:
        _CACHE["nc"] = _build_nc()
    return _CACHE["nc"]


def _host_weights(W_real, W_imag):
    """F [128,96], G2 [96,128], Wmid [24,128,128] (float32, pre-bf16)."""
    t = np.arange(B).astype(np.float64)
    F = np.zeros((128, 96))
    for fl in range(2):
        for p in range(2):
            for e in range(NP):
                f = 2 * e + fl
                col = fl * 48 + p * 24 + e
                w = 2 * np.pi * f * t / B
                F[:, col] = np.cos(w) if p == 0 else -np.sin(w)
    G2 = np.zeros((96, 128))
    scale = np.full(KT, 2.0 / B)
    scale[0] = 1.0 / B
    for fl in range(2):
        for q in range(2):
            for e in range(NP):
                f = 2 * e + fl
                w = 2 * np.pi * f * np.arange(B) / B
                G2[fl * 48 + q * 24 + e] = (
                    scale[f] * np.cos(w) if q == 0 else -scale[f] * np.sin(w))
    Wr = W_real.astype(np.float64)
    Wi = W_imag.astype(np.float64)
    Wm = np.zeros((NP, 128, 128))
    for e in range(NP):
        for fl in range(2):
            f = 2 * e + fl
            r0 = fl * 64
            Wrf = Wr[:, :, f].T
            Wif = Wi[:, :, f].T
            Wm[e, r0:r0 + 32, r0:r0 + 32] = Wrf
            Wm[e, r0 + 32:r0 + 64, r0:r0 + 32] = Wif
            Wm[e, r0:r0 + 32, r0 + 32:r0 + 64] = -Wif
            Wm[e, r0 + 32:r0 + 64, r0 + 32:r0 + 64] = Wrf
    return (F.astype(np.float32), G2.astype(np.float32),
            Wm.astype(np.float32))


def _pack_inputs(x, W_real, W_imag):
    bf16 = ml_dtypes.bfloat16
    F, G2, Wm = _host_weights(np.asarray(W_real), np.asarray(W_imag))
    wfg = np.zeros((128, WCOL_FG), np.float32)
    wfg[:, :96] = F
    wfg[:96, 96:] = G2
    wmid = Wm.transpose(1, 0, 2).reshape(128, WCOL_W)
    xt = np.ascontiguousarray(np.asarray(x, np.float32).T.astype(bf16))
    return xt, wfg.astype(bf16), np.ascontiguousarray(wmid.astype(bf16))


def kernel(x, W_real, W_imag):
    global LAST_RESULTS
    from concourse.bass_utils import run_bass_kernel_spmd

    xt, wfg, wmid = _pack_inputs(x, W_real, W_imag)
    in_maps = []
    for core in range(N_CORES):
        in_maps.append({
            "xT": np.ascontiguousarray(xt[:, core * BQ:(core + 1) * BQ]),
            "wfg": wfg,
            "wmid": wmid,
        })

    nc = _get_nc()
    res = run_bass_kernel_spmd(nc, in_maps, list(range(N_CORES)), trace=TRACE)
    LAST_RESULTS = res

    out = np.empty((BATCH, OUT_F), np.float32)
    for core in range(N_CORES):
        out[core * BQ:(core + 1) * BQ, :] = \
            res.results[core]["oT"].T.astype(np.float32)
    return out
